# revision 77
# baseline (speedup 1.0000x reference)
"""Causal self-attention (B=2, T=2048, D=768, H=12) on 8 TRN2 cores.

Sharding: core r handles batch b=r%2 and head-group g=r//2 (3 heads).
No collectives anywhere: each core writes its partial projection
y_part = O(3 heads) @ Wp_part straight to DRAM and the host-side
assemble() sums the four head-group partials per batch in fp32 (the
unshard step).  Cores never synchronize, so per-core boot skew cannot
inflate the max-core exec time.

All inputs are host-swizzled partition-major so every tensor arrives in
a handful of contiguous-descriptor dma_starts; block-0 x and the qkv
weights ride the critical path in chunk-sized calls while blocks 1-3
(2.4 MB) are issued from the back of the gpsimd queue.

Per query block bi (512 tokens), fully pipelined:
  1. qkv projection for the block (this core's 3 heads), PSUM->SBUF
     casts split across the scalar and vector queues; the v-loop
     accumulates in its own single-bank pool (ps_v) so it never rotates
     against the S pipeline.
  2. attention: S^T = K.Q with keys on partitions, heads 0/1 sharing
     one [128,1024] PSUM strip (one exp ACT covers both; the two K=64
     matmuls hit disjoint PE row groups and run concurrently), head 2
     pairing consecutive j-tiles the same way.  The AV matmul for
     j-tile tj-1 is emitted after the S matmuls for tj so the in-order
     PE never waits on the scalar-engine exp.  exp(S/8) without
     max-subtraction; on diagonal tiles a single ACT spans both head
     strips (the inter-strip span is stale PSUM whose exp lands in an
     unread region of e), halving the ACT instruction count in the
     exp-paced sections.  Denominators via ones columns appended to V
     (O^T = V_aug^T E): one ones-row for heads 0/1 (M=65 AV matmuls),
     64 replicated ones-rows for head 2 (M=128), which lands head 2's
     denominator broadcast across PSUM rows 64:128.
  3. normalize: heads 0/1 use the reciprocal+partition_broadcast chain
     (fully overlapped with the head-2 section); head 2 - the exposed,
     block-boundary chain - collapses to copy+reciprocal+multiply.
     The next block's qkv m0/m1 matmuls are emitted BEFORE the final
     AV2 flush so the PE chews on them while the last exps retire (m1
     accumulates in ps_v where its WAR is long settled).
  4. partial proj: per 128-token tile, OT01 matmuls of a tt-pair run
     before the OT2 ones (hides the o2 normalize chain); casts split
     vector/scalar; output DMA halves leave on the sync/gpsimd queues
     as soon as their own cast lands.
"""

import numpy as np

import concourse.bass as bass
import concourse.bacc as bacc
import concourse.mybir as mybir
import concourse.tile as tile
from concourse.bass_utils import run_bass_kernel_spmd

F32 = mybir.dt.float32
F16 = mybir.dt.float16

B, T, D = 2, 2048, 768
H, DH = 12, 64
NCORES = 8
HPC = H // 4          # heads per core = 3
QK = HPC * DH         # 192 rows of q (or k) per core
KC = D // 128         # 6 contraction chunks
NBI = T // 512        # 4 query blocks

EXP_SCALE = 1.0 / np.sqrt(DH)  # 0.125
# v_aug column strides.  Heads 0/1: 64 v dims + one ones column (M=65 AV
# matmuls are cheaper; their normalize chain overlaps the head-2 section).
# Head 2: 64 v dims + 64 replicated ones columns, so its AV lands the
# denominator broadcast across PSUM rows 64:128 and the (exposed, block-
# boundary) normalize collapses to copy+reciprocal+multiply with no
# gpsimd partition_broadcast in the chain.
VW01 = 65
VW2 = 128


def _emit(tc, aps):
    nc = tc.nc
    xT, wqkT, wvT, wpT, triu, y = (
        aps["xT"], aps["wqkT"], aps["wvT"], aps["wpT"], aps["triu"], aps["y"])

    pools = []

    def pool(name, bufs, space="SBUF"):
        p = tc.tile_pool(name=name, bufs=bufs, space=space)
        pools.append(p)
        return p.__enter__()

    consts = pool("consts", 1)
    xw = pool("xw", 1)
    qk_sb = pool("qk_sb", 1)
    v_sb = pool("v_sb", 1)
    work = pool("work", 3)
    norm = pool("norm", 2)
    ot_sb = pool("ot_sb", 2)
    ysb = pool("ysb", 4)
    ps = pool("ps", 2, space="PSUM")
    ps_o = pool("ps_o", 1, space="PSUM")
    ps_v = pool("ps_v", 1, space="PSUM")

    # ---- input loads: everything host-swizzled to partition-major so each
    # tensor arrives in ONE dma_start (descriptor streams stay contiguous and
    # the per-call issue cost on the engine queues collapses).
    # x layout: [128, NBI*KC*512]; col (bi*KC+k)*512+t = x[b][bi*512+t, k*128+p]
    x_sb = xw.tile([128, NBI * KC * 512], F16, tag="x", name="x")
    wqk_sb = consts.tile([128, KC * 2 * QK], F16, tag="wqk", name="wqk")
    wv_sb = consts.tile([128, KC * QK], F16, tag="wv", name="wv")

    def xcol(bi, k):  # column base of (block, k-chunk) in x_sb
        return (bi * KC + k) * 512

    # critical path first: wqk k-chunk 0 + per-chunk x(b0) calls alternating
    # sync/scalar, so chunk k lands k-th and the k-outer first-block qkv
    # consumes each on arrival; x blocks 1-3 are issued from the BACK of the
    # gpsimd queue (after the boot memsets) so their 2.4 MB never competes
    # with block-0 latency.
    nc.gpsimd.dma_start(wqk_sb[:, 0:2 * QK], wqkT[:, 0:2 * QK])
    nc.sync.dma_start(x_sb[:, 0:512], xT[:, 0:512])
    nc.scalar.dma_start(x_sb[:, 512:1024], xT[:, 512:1024])
    nc.gpsimd.dma_start(wqk_sb[:, 2 * QK:], wqkT[:, 2 * QK:])
    for k in range(2, KC):
        (nc.sync if k % 2 == 0 else nc.scalar).dma_start(
            x_sb[:, k * 512:(k + 1) * 512], xT[:, k * 512:(k + 1) * 512])
    triu_sb = consts.tile([128, 128], F16, tag="triu", name="triu")
    nc.gpsimd.dma_start(wv_sb[:], wvT[:, :])
    nc.gpsimd.dma_start(triu_sb[:], triu[:, :])
    wpA_sb = consts.tile([128, D], F16, tag="wpA", name="wpA")
    wpB_sb = consts.tile([64, D], F16, tag="wpB", name="wpB")

    # ---- persistent q/k/v tiles ----
    # heads 0/1 packed into [128, T] (rows 0-63 / 64-127); head 2 in [64, T].
    qTp = qk_sb.tile([128, T], F16, tag="qTp", name="qTp")
    kTp = qk_sb.tile([128, T], F16, tag="kTp", name="kTp")
    # head 2 q/k live in BOTH partition halves: paired j-tiles then hit
    # disjoint PE row groups (h0/h64) and run concurrently, like heads 0/1.
    qT2 = qk_sb.tile([128, T], F16, tag="qT2", name="qT2")
    kT2 = qk_sb.tile([128, T], F16, tag="kT2", name="kT2")
    VWS = [VW01, VW01, VW2]
    v_aug = [v_sb.tile([128, (T // 128) * VWS[h]], F16, tag=f"v{h}", name=f"v{h}")
             for h in range(HPC)]
    # static ones columns, set once at boot; only the v-data halves are
    # rewritten per block.  Emitted on the gpsimd queue ahead of the
    # deferred x DMAs below.
    for h in range(HPC):
        vw = VWS[h]
        for tt in range(T // 128):
            nc.gpsimd.memset(v_aug[h][:, tt * vw + 64:(tt + 1) * vw], 1.0)
    # deferred bulk loads ride the back of the gpsimd queue: x blocks 1-3
    # with the proj weights (first needed ~25us in) slotted between
    nc.gpsimd.dma_start(
        x_sb[:, xcol(1, 0):xcol(2, 0)], xT[:, xcol(1, 0):xcol(2, 0)])
    nc.gpsimd.dma_start(wpA_sb[:], wpT[0:128, :])
    nc.gpsimd.dma_start(wpB_sb[:], wpT[128:QK, :])
    for bi in range(2, NBI):
        nc.gpsimd.dma_start(
            x_sb[:, xcol(bi, 0):xcol(bi + 1, 0)], xT[:, xcol(bi, 0):xcol(bi + 1, 0)])

    def _qk_mm(bi, m):
        p = ps.tile([128, 1024], F32, tag="s", name="qkps")[:, 0:512]
        for k in range(KC):
            nc.tensor.matmul(
                p[:],
                wqk_sb[:, k * 2 * QK + m * 128:k * 2 * QK + (m + 1) * 128],
                x_sb[:, xcol(bi, k):xcol(bi, k) + 512],
                start=(k == 0), stop=(k == KC - 1))
        return p

    def emit_qkv_m0(bi):
        # m0's cast rides scalar, so emitting it before the last AV2 flush
        # leaves the vector queue free for the triu mask that gates AV2.
        ns = slice(bi * 512, (bi + 1) * 512)
        p = _qk_mm(bi, 0)
        nc.scalar.copy(qTp[:, ns], p[:])

    def emit_qkv_m1_mm(bi):
        # m1 accumulates in the ps_v bank: unlike the rotating ps pool, its
        # WAR is on the long-done v casts, so these matmuls can run during
        # the S2 exp drain when emitted before the final AV2 flush.
        p = ps_v.tile([128, 512], F32, tag="v", name="m1ps")
        for k in range(KC):
            nc.tensor.matmul(
                p[:],
                wqk_sb[:, k * 2 * QK + 128:k * 2 * QK + 256],
                x_sb[:, xcol(bi, k):xcol(bi, k) + 512],
                start=(k == 0), stop=(k == KC - 1))
        return p

    def emit_qkv_m12_rest(bi, p1):
        ns = slice(bi * 512, (bi + 1) * 512)
        nc.vector.tensor_copy(qT2[0:64, ns], p1[0:64, :])
        nc.vector.tensor_copy(qT2[64:128, ns], p1[0:64, :])
        nc.vector.tensor_copy(kTp[0:64, ns], p1[64:128, :])
        p = _qk_mm(bi, 2)
        nc.scalar.copy(kTp[64:128, ns], p[0:64, :])
        nc.scalar.copy(kT2[0:64, ns], p[64:128, :])
        nc.scalar.copy(kT2[64:128, ns], p[64:128, :])

    def emit_qkv_v(bi):
        for tt in range(bi * 4, bi * 4 + 4):
            # own single-bank pool, two region slots: never contends with the
            # m-strip casts for PSUM write-after-read
            p = ps_v.tile([128, 512], F32, tag="v", name="vps")[
                :, (tt % 2) * 256:(tt % 2) * 256 + QK]
            u = tt % 4
            for k in range(KC):
                nc.tensor.matmul(
                    p[:],
                    x_sb[:, xcol(bi, k) + u * 128:xcol(bi, k) + (u + 1) * 128],
                    wv_sb[:, k * QK:(k + 1) * QK],
                    start=(k == 0), stop=(k == KC - 1))
            for h in range(HPC):
                nc.vector.tensor_copy(
                    v_aug[h][:, tt * VWS[h]:tt * VWS[h] + 64],
                    p[:, h * 64:(h + 1) * 64])

    def normalize01(o_ps, dst, i):
        # heads 0/1: single denominator row.  Stays entirely off the scalar
        # queue (exp must not sit behind these) and overlaps the head-2
        # section / next-block qkv on the PE.
        den = norm.tile([1, 512], F32, tag=f"den{i}", name=f"den{i}")
        nc.vector.tensor_copy(den[:], o_ps[64:65, :])
        rec = norm.tile([1, 512], F32, tag=f"rec{i}", name=f"rec{i}")
        # den > 0 always (the exp terms are positive), so approx_fast is safe
        nc.vector.reciprocal_approx_fast(rec[:], den[:])
        rb = norm.tile([64, 512], F32, tag=f"rb{i}", name=f"rb{i}")
        nc.gpsimd.partition_broadcast(rb[:], rec[:])
        nc.vector.tensor_mul(dst, o_ps[0:64, :], rb[:])

    def normalize2(o_ps, dst):
        # head 2: denominator already replicated across PSUM rows 64:128.
        # The copy rides scalar (free right after the last exp) so only
        # reciprocal+multiply occupy the vector queue.
        den = norm.tile([64, 512], F32, tag="den2", name="den2")
        nc.scalar.copy(den[:], o_ps[64:128, :])
        rec = norm.tile([64, 512], F32, tag="rec2", name="rec2")
        nc.vector.reciprocal_approx_fast(rec[:], den[:])
        nc.vector.tensor_mul(dst, o_ps[0:64, :], rec[:])

    emit_qkv_m0(0)
    emit_qkv_m12_rest(0, emit_qkv_m1_mm(0))
    emit_qkv_v(0)
    for bi in range(NBI):
        ntj = 4 * bi + 4
        o01 = [ps_o.tile([65, 512], F32, tag=f"o{h}", name=f"o{h}") for h in range(2)]
        o2 = ps_o.tile([128, 512], F32, tag="o2", name="o2")
        OT01 = ot_sb.tile([128, 512], F16, tag="OT01", name="OT01")
        OT2 = ot_sb.tile([64, 512], F16, tag="OT2", name="OT2")

        # ---- heads 0/1: S(tj) then AV(tj-1), one exp per j-tile ----
        pend = None  # (e_tile, tj, lo)

        def flush_av():
            e, tj, lo = pend
            for h in range(2):
                if tj - 4 * bi >= 0:
                    nc.vector.tensor_mul(
                        e[:, h * 512 + lo:h * 512 + lo + 128],
                        e[:, h * 512 + lo:h * 512 + lo + 128], triu_sb[:])
                nc.tensor.matmul(
                    o01[h][:, lo:],
                    v_aug[h][:, tj * VW01:(tj + 1) * VW01],
                    e[:, h * 512 + lo:(h + 1) * 512],
                    start=(tj == 0), stop=(tj == ntj - 1))

        for tj in range(ntj):
            dtile = tj - 4 * bi
            lo = max(dtile, 0) * 128
            js = slice(tj * 128, (tj + 1) * 128)
            qs = slice(bi * 512 + lo, (bi + 1) * 512)
            s_ps = ps.tile([128, 1024], F32, tag="s", name="s")
            nc.tensor.matmul(s_ps[:, lo:512], kTp[0:64, js], qTp[0:64, qs],
                             start=True, stop=True)
            nc.tensor.matmul(s_ps[:, 512 + lo:1024], kTp[64:128, js], qTp[64:128, qs],
                             start=True, stop=True)
            e = work.tile([128, 1024], F16, tag="e", name="e")
            # ONE ACT spanning both head strips even on diagonal tiles: the
            # [512:512+lo) span between them is stale PSUM whose exp lands
            # in a region of e no AV matmul reads; merging drops the second
            # ACT's ~380ns fixed cost from the scalar queue that paces this
            # section.
            nc.scalar.activation(e[:, lo:1024], s_ps[:, lo:1024],
                                 mybir.ActivationFunctionType.Exp, scale=EXP_SCALE)
            if pend is not None:
                flush_av()
            pend = (e, tj, lo)

        # ---- head 2: paired j-tiles, AV one pair behind ----
        def flush_av2(ep, pp, losp):
            for idx, tj in enumerate(pp):
                if tj - 4 * bi >= 0:
                    nc.vector.tensor_mul(
                        ep[:, idx * 512 + losp[idx]:idx * 512 + losp[idx] + 128],
                        ep[:, idx * 512 + losp[idx]:idx * 512 + losp[idx] + 128],
                        triu_sb[:])
                nc.tensor.matmul(
                    o2[:, losp[idx]:],
                    v_aug[2][:, tj * VW2:(tj + 1) * VW2],
                    ep[:, idx * 512 + losp[idx]:(idx + 1) * 512],
                    start=(tj == 0), stop=(tj == ntj - 1))

        def s2_pair(tj0):
            pair = (tj0, tj0 + 1)
            s_ps = ps.tile([128, 1024], F32, tag="s", name="s2")
            e = work.tile([128, 1024], F16, tag="e", name="e2")
            los = []
            for idx, tj in enumerate(pair):
                lo = max(tj - 4 * bi, 0) * 128
                los.append(lo)
                hs = slice(64 * idx, 64 * idx + 64)
                js = slice(tj * 128, (tj + 1) * 128)
                qs = slice(bi * 512 + lo, (bi + 1) * 512)
                nc.tensor.matmul(
                    s_ps[:, idx * 512 + lo:(idx + 1) * 512],
                    kT2[hs, js], qT2[hs, qs],
                    start=True, stop=True)
            # merged ACT as in the heads-0/1 loop: [512:512+los1) is stale
            # PSUM, exp'd into an unread region of e
            nc.scalar.activation(e[:, los[0]:1024], s_ps[:, los[0]:1024],
                                 mybir.ActivationFunctionType.Exp, scale=EXP_SCALE)
            return (e, pair, los)

        # the first S2 pair is hoisted ahead of the final AV01 flush, so the
        # PE streams it while the last S01 exp retires instead of stalling
        # on the flush's exp dependency
        pend2 = s2_pair(0)
        flush_av()
        if bi == NBI - 1:
            # last block: o01 chains start now, overlapping the S2 section,
            # so only the short o2 chain is exposed before the final proj
            normalize01(o01[0], OT01[0:64], 0)
            normalize01(o01[1], OT01[64:128], 1)
        for tj0 in range(2, ntj, 2):
            nxt = s2_pair(tj0)
            flush_av2(*pend2)
            pend2 = nxt
        # drain: the next block's qkv m0+m1 matmuls are emitted BEFORE the
        # final AV2 flush so the PE chews ~2.6us while the last two exps
        # retire (their casts stay after the flush so the vector-queue triu
        # mask that gates AV2 isn't delayed).
        p1 = None
        if bi + 1 < NBI:
            emit_qkv_m0(bi + 1)
            p1 = emit_qkv_m1_mm(bi + 1)
        flush_av2(*pend2)
        if bi + 1 < NBI:
            emit_qkv_m12_rest(bi + 1, p1)
        # normalize chains ahead of the v-loop casts on the vector queue:
        # proj (gated by the muls) is due sooner than v_aug (next block's
        # diagonal AVs, ~10us away).
        if bi < NBI - 1:
            normalize01(o01[0], OT01[0:64], 0)
            normalize01(o01[1], OT01[64:128], 1)
        normalize2(o2, OT2[:, :])
        if bi + 1 < NBI:
            emit_qkv_v(bi + 1)

        # ---- partial proj, straight to DRAM (host sums the partials) ----
        # tt pairs share the two rotating ps buffers; all four OT01 matmuls
        # of a pair run before the OT2 ones, hiding the o2 normalize chain
        # behind real PE work (matters for the exposed final block).
        # Casts split across vector+scalar; output DMA halves leave as soon
        # as their own cast lands (sync / gpsimd queues).
        for tp in range(2):
            pjs = []
            for tt in (2 * tp, 2 * tp + 1):
                ts = slice(tt * 128, (tt + 1) * 128)
                pj = ps.tile([128, 1024], F32, tag="s", name="pj")
                pjs.append(pj)
                # matmul outputs may not cross a PSUM bank: 512 cols in
                # bank 0, the remaining 256 in bank 1 of the same tile.
                for on, osz in ((0, 512), (512, 256)):
                    nc.tensor.matmul(
                        pj[:, on:on + osz], OT01[:, ts], wpA_sb[:, on:on + osz],
                        start=True, stop=False)
            for tt in (2 * tp, 2 * tp + 1):
                ts = slice(tt * 128, (tt + 1) * 128)
                pj = pjs[tt - 2 * tp]
                for on, osz in ((0, 512), (512, 256)):
                    nc.tensor.matmul(
                        pj[:, on:on + osz], OT2[:, ts], wpB_sb[:, on:on + osz],
                        start=False, stop=True)
                y_t = ysb.tile([128, D], F16, tag="y_t", name="y_t")
                nc.vector.tensor_copy(y_t[:, 0:384], pj[:, 0:384])
                nc.scalar.copy(y_t[:, 384:D], pj[:, 384:D])
                yrow = slice(bi * 512 + tt * 128, bi * 512 + (tt + 1) * 128)
                nc.sync.dma_start(y[yrow, 0:384], y_t[:, 0:384])
                last = (bi == NBI - 1 and tt == 3)
                (nc.scalar if last else nc.gpsimd).dma_start(
                    y[yrow, 384:D], y_t[:, 384:D])

    for p in reversed(pools):
        p.__exit__(None, None, None)


_NC_CACHE = {}


def _get_nc():
    if "nc" in _NC_CACHE:
        return _NC_CACHE["nc"]
    nc = bacc.Bacc("TRN2", num_devices=NCORES, debug=False)
    aps = {
        "xT": nc.dram_tensor(
            "xT", [128, NBI * KC * 512], F16, kind="ExternalInput").ap(),
        "wqkT": nc.dram_tensor(
            "wqkT", [128, KC * 2 * QK], F16, kind="ExternalInput").ap(),
        "wvT": nc.dram_tensor(
            "wvT", [128, KC * QK], F16, kind="ExternalInput").ap(),
        "wpT": nc.dram_tensor("wpT", [QK, D], F16, kind="ExternalInput").ap(),
        "triu": nc.dram_tensor("triu", [128, 128], F16, kind="ExternalInput").ap(),
        "y": nc.dram_tensor("y", [T, D], F16, kind="ExternalOutput").ap(),
    }
    with tile.TileContext(nc, num_cores=NCORES) as tc:
        _emit(tc, aps)
    nc.compile()
    _NC_CACHE["nc"] = nc
    return nc


def make_in_maps(x, W_qkv, W_proj):
    triu = np.triu(np.ones((128, 128), dtype=np.float16))
    wpT_full = np.ascontiguousarray(W_proj.T).astype(np.float16)  # [in, out]
    in_maps = []
    for r in range(NCORES):
        b, g = r % 2, r // 2
        rs = slice(QK * g, QK * (g + 1))
        wq = W_qkv[0:D][rs]
        wk = W_qkv[D:2 * D][rs]
        wv = W_qkv[2 * D:3 * D][rs]
        # partition-major folds: row p holds k-chunk blocks back to back, so
        # each tensor arrives in a single contiguous-descriptor dma_start.
        wqkT = np.ascontiguousarray(
            np.concatenate([wq, wk], axis=0).T.astype(np.float16)
            .reshape(KC, 128, 2 * QK).transpose(1, 0, 2).reshape(128, KC * 2 * QK))
        wvT = np.ascontiguousarray(
            wv.T.astype(np.float16)
            .reshape(KC, 128, QK).transpose(1, 0, 2).reshape(128, KC * QK))
        wpT = np.ascontiguousarray(wpT_full[rs, :])
        # x: [128, NBI*KC*512]; col (bi*KC+k)*512+t = x[b][bi*512+t, k*128+p]
        xT = np.ascontiguousarray(
            x[b].astype(np.float16).reshape(NBI, 512, KC, 128)
            .transpose(3, 0, 2, 1).reshape(128, NBI * KC * 512))
        in_maps.append({"xT": xT, "wqkT": wqkT, "wvT": wvT,
                        "wpT": wpT, "triu": triu})
    return in_maps


def assemble(results):
    # unshard: per batch, sum the four head-group partial projections
    y = np.zeros((B, T, D), dtype=np.float32)
    for r in range(NCORES):
        b = r % 2
        y[b] += results[r]["y"].astype(np.float32)
    return y


def kernel(**inputs):
    x = np.asarray(inputs["x"], dtype=np.float32)
    W_qkv = np.asarray(inputs["W_qkv"], dtype=np.float32)
    W_proj = np.asarray(inputs["W_proj"], dtype=np.float32)
    nc = _get_nc()
    in_maps = make_in_maps(x, W_qkv, W_proj)
    res = run_bass_kernel_spmd(nc, in_maps, core_ids=list(range(NCORES)))
    return assemble(res.results)

